# revision 31
# baseline (speedup 1.0000x reference)
"""Trainium2 Bass kernel for CalibConv (5x5 conv -> linear -> drift modulation).

Math: per kernel position p=(i,j) and class k:
    cmap[n,h,w,p,k] = sum_c x_pad[n,c,h+i,w+j] * Weff[k,c,p] + beff[k]
with Weff[k,c,p] = sum_o Wlin[k,o]*Wt[o,c,i,j], beff = Wlin@bias + blin.
Per output pixel: asum = sum_p |cmap|, ysum = sum_p yofs_p*|cmap|, xsum
likewise, csum = sum_p cmap, out = csum * exp(-0.5*sqrt(ysum^2+xsum^2)/asum).

Device strategy (one batch element per core; bf16 internals):
  * One manual InstLoadActFuncSet(natural_log_exp_and_others) covers every
    ACT function used (Prelu, Square, Ln, Exp) -> zero table switches.
  * ~24 tiny junk matmuls warm the PE HAM clock gate while x loads.
  * G phase: row layout 32*u + p with u = (sign k0, sign k1, relu k0,
    relu k1); the relu blocks hold NEGATED weights/bias, using
    |v| = v + 2*relu(-v) so the abs branch is a plain add+max(0).
    DVE handles chunks 0,1,7 (two tensor_scalar ops each); ACT handles
    2..6 as Prelu superchunks (alpha=1 sign rows, alpha=0 relu rows),
    each aligned with the arrival of one 900-col input DMA.
  * gather: 25 SBUF->SBUF DMAs, one per position p, via partition
    step-slices bc[p::32] <- ag[p::32] shifted by s_p = 60i+j. A DMA AP
    supports exactly one (strided) partition dim; step-slicing keeps the
    tile framework's dependency tracking intact. DMA issue cost is
    ~0.6us flat per dma_start, so issue count dominates this phase;
    they are spread over the two HWDGE queues + the SWDGE queue.
  * stats: 28 chunks of 120 positions (2 padded rows); stationary
    bc[:, off:off+128] (128 cols -> Fast Weight Load), moving smat
    [128, 8] -> psum [pos, chunk, stat]; ~30ns/matmul pipelined.
  * epilogue: reciprocal (DVE), Square (ACT), add, then
    sqrt(s) = exp(0.5*ln(s)) on ACT (stays in the one table set),
    Exp(scale=-0.5), final muls; two output DMAs on both HWDGE queues.
    Host unscrambles [120, 28, 2] -> [56, 56, 2].
"""

import numpy as np
import ml_dtypes

import concourse.bacc as bacc
import concourse.mybir as mybir
from concourse import tile
from concourse.bass_utils import run_bass_kernel_spmd

N_CORES = 8
C, H, W = 128, 56, 56
KS, PAD = 5, 2
HP, WP = H + 2 * PAD, W + 2 * PAD  # 60, 60
NPIX = HP * WP                      # 3600
GLEN = (H - 1) * WP + W             # 3356 gather length per row
BCW = 3376                          # gathered width (28 chunks of 120) + FWL pad
AB = 64                             # relu-branch base row (DVE needs 32-aligned)
NR = 128                            # row u*32+p: u in (sign k0, sign k1, relu k0, relu k1)
NCLS = 2

G_CHUNK = 450
N_GCHUNK = NPIX // G_CHUNK          # 8
S_CHUNK = 2 * WP                    # 120
N_SCHUNK = BCW // S_CHUNK           # 28

F32 = mybir.dt.float32
BF16 = mybir.dt.bfloat16
I32 = mybir.dt.int32
AF = mybir.ActivationFunctionType
ALU = mybir.AluOpType


def kernel_body(tc, x_d, wp_d, actp_d, out_d):
    nc = tc.nc
    with (
        tc.tile_pool(name="const", bufs=1) as cpool,
        tc.tile_pool(name="big", bufs=1) as bpool,
        tc.tile_pool(name="psa", bufs=3, space="PSUM") as psa_pool,
        tc.tile_pool(name="psb", bufs=2, space="PSUM") as psb_pool,
        tc.tile_pool(name="tmp", bufs=1) as tpool,
    ):
        wp_sb = cpool.tile([C, NR + 8], BF16)
        actp_sb = cpool.tile([NR, 2], F32)

        # preload the one ACT table set covering every function we use
        # (natural_log_exp_and_others: Prelu, Square, Ln, Exp) -> the
        # membership-based auto-pass then inserts no further loads
        nc.scalar.add_instruction(mybir.InstLoadActFuncSet(
            name=nc.get_next_instruction_name(), act_func_set_id=6))

        # params gate LDWEIGHTS/branch ops; x follows in 4 chunks
        # alternating across the two HWDGE queues
        xp = bpool.tile([C, NPIX], BF16)
        XCH = NPIX // 4
        for b in range(2):
            eng = nc.sync if b % 2 == 0 else nc.scalar
            eng.dma_start(
                xp[:, XCH * b : XCH * (b + 1)],
                x_d[:, XCH * b : XCH * (b + 1)],
            )
        nc.sync.dma_start(wp_sb[:], wp_d[:])
        nc.scalar.dma_start(actp_sb[:], actp_d[:])
        for b in range(2, 4):
            eng = nc.sync if b % 2 == 0 else nc.scalar
            eng.dma_start(
                xp[:, XCH * b : XCH * (b + 1)],
                x_d[:, XCH * b : XCH * (b + 1)],
            )

        # PE warm-up: dummy matmuls on a junk tile while the input loads,
        # so the HAM clock gate reaches 8/8 before the real G matmuls
        junk = cpool.tile([C, 640], BF16)
        nc.vector.memset(junk[:], 1.0)
        ps_warm = psb_pool.tile([C, 450], F32, tag="psb")
        for _ in range(28):
            nc.tensor.matmul(
                ps_warm[:, 0:128], junk[:, 0:C], junk[:, 128:256],
                start=True, stop=True,
            )

        ag = bpool.tile([NR, NPIX], BF16)
        bc = bpool.tile([128, BCW], BF16)
        nc.gpsimd.memset(bc[:], 0.0)

        # G phase. DVE handles chunks 0,1 (earliest data) and chunk 7 (the
        # last to arrive); ACT handles 2..6 as two 900-col Prelu superchunks
        # plus one 450-col chunk, aligned with input-chunk arrival.
        for g in (0, 1, 7):
            sl = slice(G_CHUNK * g, G_CHUNK * (g + 1))
            ps = psb_pool.tile([NR, G_CHUNK], F32, tag="psb")
            nc.tensor.matmul(ps[:], wp_sb[:, 0:NR], xp[:, sl], start=True, stop=True)
            nc.vector.tensor_scalar(
                ag[0:AB, sl], ps[0:AB, :], actp_sb[0:AB, 0:1], None, ALU.add
            )
            nc.vector.tensor_scalar(
                ag[AB:NR, sl], ps[AB:NR, :], actp_sb[AB:NR, 0:1], 0.0,
                ALU.add, ALU.max,
            )
        for s in range(3):
            g0 = 2 + 2 * s
            nch = 2 if s < 2 else 1
            sl = slice(G_CHUNK * g0, G_CHUNK * (g0 + nch))
            ps = psa_pool.tile([NR, 2, 512], F32, tag="psa")
            for h in range(nch):
                nc.tensor.matmul(
                    ps[:, h, 0:G_CHUNK],
                    wp_sb[:, 0:NR],
                    xp[:, G_CHUNK * (g0 + h) : G_CHUNK * (g0 + h + 1)],
                    start=True, stop=True,
                )
            nc.scalar.activation(
                ag[:, sl], ps[:, 0:nch, 0:G_CHUNK], AF.Prelu,
                bias=actp_sb[:, 0:1], alpha=actp_sb[:, 1:2],
            )

        # gather: 25 direct SBUF->SBUF DMAs, one per kernel position p,
        # shifting the four planes of p (partition step-slice p::32) left by
        # s_p = 60i+j. Step slices lower to a single strided partition dim,
        # which the DMA hardware supports, and keep dependency tracking.
        g_engs = [nc.gpsimd, nc.sync, nc.scalar] * 8 + [nc.sync]
        for p in range(KS * KS):
            i, j = p // KS, p % KS
            sp = WP * i + j
            g_engs[p].dma_start(
                bc[p : p + 97 : 32, 0:GLEN], ag[p : p + 97 : 32, sp : sp + GLEN],
                single_packet=True,
            )

        # stats: psum[pos, stat] = sum_r bc[r, off+pos] * smat[r, stat]
        ps_stats = psb_pool.tile([128, N_SCHUNK, 8], F32, tag='psb')
        for s in range(N_SCHUNK):
            off = S_CHUNK * s
            nc.tensor.matmul(
                ps_stats[:, s, :],
                bc[0:NR, off : off + 128],
                wp_sb[0:NR, NR : NR + 8],
                start=True, stop=True,
            )

        # epilogue: out = csum * exp(-0.5 * sqrt(ysum^2+xsum^2) / asum)
        # sqrt(s) = exp(0.5*ln(s)): Prelu/Square/Ln/Exp all live in the
        # natural_log_exp_and_others ACT table set -> no mid-kernel loads.
        rinv = tpool.tile([S_CHUNK, N_SCHUNK, NCLS], F32)
        sq = tpool.tile([S_CHUNK, N_SCHUNK, 4], F32)
        ssum = tpool.tile([S_CHUNK, N_SCHUNK, NCLS], F32)
        lns = tpool.tile([S_CHUNK, N_SCHUNK, NCLS], F32)
        srt = tpool.tile([S_CHUNK, N_SCHUNK, NCLS], F32)
        drift = tpool.tile([S_CHUNK, N_SCHUNK, NCLS], F32)
        expd = tpool.tile([S_CHUNK, N_SCHUNK, NCLS], F32)
        outv = tpool.tile([S_CHUNK, N_SCHUNK, NCLS], F32)
        nc.vector.reciprocal(rinv[:], ps_stats[0:S_CHUNK, :, 0:2])
        nc.scalar.activation(sq[:], ps_stats[0:S_CHUNK, :, 2:6], AF.Square)
        nc.vector.tensor_tensor(ssum[:], sq[:, :, 0:2], sq[:, :, 2:4], op=ALU.add)
        nc.scalar.activation(lns[:], ssum[:], AF.Ln)
        nc.scalar.activation(srt[:], lns[:], AF.Exp, scale=0.5)
        nc.vector.tensor_tensor(drift[:], srt[:], rinv[:], op=ALU.mult)
        nc.scalar.activation(expd[:], drift[:], AF.Exp, scale=-0.5)
        nc.vector.tensor_tensor(
            outv[:], ps_stats[0:S_CHUNK, :, 6:8], expd[:], op=ALU.mult
        )
        nc.sync.dma_start(out_d[:, 0 : NCLS * 14], outv[:, 0:14, :])
        nc.scalar.dma_start(out_d[:, NCLS * 14 :], outv[:, 14:, :])


def build_program():
    nc = bacc.Bacc("TRN2", target_bir_lowering=False, debug=False)
    x_d = nc.dram_tensor("x", [C, NPIX], BF16, kind="ExternalInput").ap()
    wp_d = nc.dram_tensor("wp", [C, NR + 8], BF16, kind="ExternalInput").ap()
    actp_d = nc.dram_tensor("actp", [NR, 2], F32, kind="ExternalInput").ap()
    out_d = nc.dram_tensor(
        "out", [S_CHUNK, N_SCHUNK * NCLS], F32, kind="ExternalOutput"
    ).ap()
    with tile.TileContext(nc) as tc:
        kernel_body(tc, x_d, wp_d, actp_d, out_d)
    nc.compile()
    return nc


def host_params(Wt, bias, Wlin, blin):
    """Fold conv weights + linear projection into device params."""
    Wt = np.asarray(Wt, np.float32)
    bias = np.asarray(bias, np.float32)
    Wlin = np.asarray(Wlin, np.float32)
    blin = np.asarray(blin, np.float32)
    O = Wt.shape[0]
    P25 = KS * KS
    Wp = Wt.reshape(O, C, P25)                        # (O, C, P)
    Weff = np.einsum("ko,ocp->kcp", Wlin, Wp)         # (2, C, P)
    beff = (Wlin @ bias + blin).astype(np.float32)    # (2,)
    offs = np.arange(-PAD, PAD + 1, dtype=np.float32)

    wp = np.zeros((C, NR + 8), np.float32)
    actp = np.zeros((NR, 2), np.float32)
    actp[0:AB, 1] = 1.0
    for p in range(P25):
        i, j = p // KS, p % KS
        for k in range(NCLS):
            rs = 32 * k + p            # sign row (u = k)
            rr = AB + 32 * k + p       # relu row (u = 2 + k)
            wp[:, rs] = Weff[k, :, p]
            wp[:, rr] = -Weff[k, :, p]     # negated: row holds -G'
            actp[rs, 0] = beff[k]
            actp[rr, 0] = -beff[k]
            # smat columns appended after the weight columns; smat row
            # index == ag row index. |v| = v + 2*relu(-v): sign rows carry
            # weight 1, relu rows weight 2, for the abs-based stats.
            wp[rs, NR + 6 + k] = 1.0           # csum (sign rows only)
            wp[rs, NR + 0 + k] = 1.0           # asum
            wp[rs, NR + 2 + k] = offs[i]       # ysum
            wp[rs, NR + 4 + k] = offs[j]       # xsum
            wp[rr, NR + 0 + k] = 2.0
            wp[rr, NR + 2 + k] = 2.0 * offs[i]
            wp[rr, NR + 4 + k] = 2.0 * offs[j]
    return wp.astype(ml_dtypes.bfloat16), actp


_nc_cache = None
last_results = None  # BassKernelResults of the most recent run (for profiling)


def kernel(x, Wt, bias, Wlin, blin):
    global _nc_cache, last_results
    x = np.asarray(x, np.float32)
    xpad = np.ascontiguousarray(
        np.pad(x, ((0, 0), (0, 0), (PAD, PAD), (PAD, PAD))).reshape(
            N_CORES, C, NPIX
        )
    ).astype(ml_dtypes.bfloat16)
    wp, actp = host_params(Wt, bias, Wlin, blin)
    if _nc_cache is None:
        _nc_cache = build_program()
    in_maps = [
        {"x": xpad[n], "wp": wp, "actp": actp} for n in range(N_CORES)
    ]
    res = run_bass_kernel_spmd(_nc_cache, in_maps, list(range(N_CORES)))
    last_results = res
    # device out: [120, 28, 2] = [(d, wp), h2, k]; pixel (h, w) lives at
    # partition 60*(h%2)+w, chunk h//2
    out = np.empty((N_CORES, H, W, NCLS), np.float32)
    hh = np.arange(H)
    for n in range(N_CORES):
        arr = res.results[n]["out"].reshape(S_CHUNK, N_SCHUNK, NCLS)
        # out[h, w, k] = arr[60*(h%2) + w, h//2, k]
        out[n] = arr[
            (60 * (hh % 2))[:, None] + np.arange(W)[None, :], (hh // 2)[:, None], :
        ]
    return out


# revision 32
# speedup vs baseline: 1.0152x; 1.0152x over previous
"""Trainium2 Bass kernel for CalibConv (5x5 conv -> linear -> drift modulation).

Math: per kernel position p=(i,j) and class k:
    cmap[n,h,w,p,k] = sum_c x_pad[n,c,h+i,w+j] * Weff[k,c,p] + beff[k]
with Weff[k,c,p] = sum_o Wlin[k,o]*Wt[o,c,i,j], beff = Wlin@bias + blin.
Per output pixel: asum = sum_p |cmap|, ysum = sum_p yofs_p*|cmap|, xsum
likewise, csum = sum_p cmap, out = csum * exp(-0.5*sqrt(ysum^2+xsum^2)/asum).

Device strategy (one batch element per core; bf16 internals):
  * One manual InstLoadActFuncSet(natural_log_exp_and_others) covers every
    ACT function used (Prelu, Square, Ln, Exp) -> zero table switches.
  * ~24 tiny junk matmuls warm the PE HAM clock gate while x loads.
  * G phase: row layout 32*u + p with u = (sign k0, sign k1, relu k0,
    relu k1); the relu blocks hold NEGATED weights/bias, using
    |v| = v + 2*relu(-v) so the abs branch is a plain add+max(0).
    DVE handles chunks 0,1,7 (two tensor_scalar ops each); ACT handles
    2..6 as Prelu superchunks (alpha=1 sign rows, alpha=0 relu rows),
    each aligned with the arrival of one 900-col input DMA.
  * gather: 25 SBUF->SBUF DMAs, one per position p, via partition
    step-slices bc[p::32] <- ag[p::32] shifted by s_p = 60i+j. A DMA AP
    supports exactly one (strided) partition dim; step-slicing keeps the
    tile framework's dependency tracking intact. DMA issue cost is
    ~0.6us flat per dma_start, so issue count dominates this phase;
    they are spread over the two HWDGE queues + the SWDGE queue.
  * stats: 28 chunks of 120 positions (2 padded rows); stationary
    bc[:, off:off+128] (128 cols -> Fast Weight Load), moving smat
    [128, 8] -> psum [pos, chunk, stat]; ~30ns/matmul pipelined.
  * epilogue: reciprocal (DVE), Square (ACT), add, then
    sqrt(s) = exp(0.5*ln(s)) on ACT (stays in the one table set),
    Exp(scale=-0.5), final muls; two output DMAs on both HWDGE queues.
    Host unscrambles [120, 28, 2] -> [56, 56, 2].
"""

import numpy as np
import ml_dtypes

import concourse.bacc as bacc
import concourse.mybir as mybir
from concourse import tile
from concourse.bass_utils import run_bass_kernel_spmd

N_CORES = 8
C, H, W = 128, 56, 56
KS, PAD = 5, 2
HP, WP = H + 2 * PAD, W + 2 * PAD  # 60, 60
NPIX = HP * WP                      # 3600
GLEN = (H - 1) * WP + W             # 3356 gather length per row
BCW = 3376                          # gathered width (28 chunks of 120) + FWL pad
AB = 64                             # relu-branch base row (DVE needs 32-aligned)
NR = 128                            # row u*32+p: u in (sign k0, sign k1, relu k0, relu k1)
NCLS = 2

G_CHUNK = 450
N_GCHUNK = NPIX // G_CHUNK          # 8
S_CHUNK = 2 * WP                    # 120
N_SCHUNK = BCW // S_CHUNK           # 28

F32 = mybir.dt.float32
BF16 = mybir.dt.bfloat16
I32 = mybir.dt.int32
AF = mybir.ActivationFunctionType
ALU = mybir.AluOpType


def kernel_body(tc, x_d, wp_d, actp_d, out_d):
    nc = tc.nc
    with (
        tc.tile_pool(name="const", bufs=1) as cpool,
        tc.tile_pool(name="big", bufs=1) as bpool,
        tc.tile_pool(name="psa", bufs=3, space="PSUM") as psa_pool,
        tc.tile_pool(name="psb", bufs=2, space="PSUM") as psb_pool,
        tc.tile_pool(name="tmp", bufs=1) as tpool,
    ):
        wp_sb = cpool.tile([C, NR + 8], BF16)
        actp_sb = cpool.tile([NR, 2], F32)

        # preload the one ACT table set covering every function we use
        # (natural_log_exp_and_others: Prelu, Square, Ln, Exp) -> the
        # membership-based auto-pass then inserts no further loads
        nc.scalar.add_instruction(mybir.InstLoadActFuncSet(
            name=nc.get_next_instruction_name(), act_func_set_id=6))

        # params gate LDWEIGHTS/branch ops; x follows in 4 chunks
        # alternating across the two HWDGE queues
        xp = bpool.tile([C, NPIX], BF16)
        XCH = NPIX // 4
        for b in range(2):
            eng = nc.sync if b % 2 == 0 else nc.scalar
            eng.dma_start(
                xp[:, XCH * b : XCH * (b + 1)],
                x_d[:, XCH * b : XCH * (b + 1)],
            )
        nc.sync.dma_start(wp_sb[:], wp_d[:])
        nc.scalar.dma_start(actp_sb[:], actp_d[:])
        for b in range(2, 4):
            eng = nc.sync if b % 2 == 0 else nc.scalar
            eng.dma_start(
                xp[:, XCH * b : XCH * (b + 1)],
                x_d[:, XCH * b : XCH * (b + 1)],
            )

        # PE warm-up: dummy matmuls on a junk tile while the input loads,
        # so the HAM clock gate reaches 8/8 before the real G matmuls
        junk = cpool.tile([C, 640], BF16)
        nc.vector.memset(junk[:], 1.0)
        ps_warm = psb_pool.tile([C, 450], F32, tag="psb")
        for _ in range(28):
            nc.tensor.matmul(
                ps_warm[:, 0:128], junk[:, 0:C], junk[:, 128:256],
                start=True, stop=True,
            )

        ag = bpool.tile([NR, NPIX], BF16)
        bc = bpool.tile([128, BCW], BF16)
        nc.gpsimd.memset(bc[:], 0.0)

        # G phase. DVE handles chunks 0,1 (earliest data) and chunk 7 (the
        # last to arrive); ACT handles 2..6 as two 900-col Prelu superchunks
        # plus one 450-col chunk, aligned with input-chunk arrival.
        for g in (0, 1, 7):
            sl = slice(G_CHUNK * g, G_CHUNK * (g + 1))
            ps = psb_pool.tile([NR, G_CHUNK], F32, tag="psb")
            nc.tensor.matmul(ps[:], wp_sb[:, 0:NR], xp[:, sl], start=True, stop=True)
            nc.vector.tensor_scalar(
                ag[0:AB, sl], ps[0:AB, :], actp_sb[0:AB, 0:1], None, ALU.add
            )
            nc.vector.tensor_scalar(
                ag[AB:NR, sl], ps[AB:NR, :], actp_sb[AB:NR, 0:1], 0.0,
                ALU.add, ALU.max,
            )
        for s in range(3):
            g0 = 2 + 2 * s
            nch = 2 if s < 2 else 1
            sl = slice(G_CHUNK * g0, G_CHUNK * (g0 + nch))
            ps = psa_pool.tile([NR, 2, 512], F32, tag="psa")
            for h in range(nch):
                nc.tensor.matmul(
                    ps[:, h, 0:G_CHUNK],
                    wp_sb[:, 0:NR],
                    xp[:, G_CHUNK * (g0 + h) : G_CHUNK * (g0 + h + 1)],
                    start=True, stop=True,
                )
            nc.scalar.activation(
                ag[:, sl], ps[:, 0:nch, 0:G_CHUNK], AF.Prelu,
                bias=actp_sb[:, 0:1], alpha=actp_sb[:, 1:2],
            )

        # gather: 25 direct SBUF->SBUF DMAs, one per kernel position p,
        # shifting the four planes of p (partition step-slice p::32) left by
        # s_p = 60i+j. Step slices lower to a single strided partition dim,
        # which the DMA hardware supports, and keep dependency tracking.
        g_engs = [nc.gpsimd, nc.sync, nc.scalar] * 8 + [nc.sync]
        for p in range(KS * KS):
            i, j = p // KS, p % KS
            sp = WP * i + j
            g_engs[p].dma_start(
                bc[p : p + 97 : 32, 0:GLEN], ag[p : p + 97 : 32, sp : sp + GLEN]
            )

        # stats: psum[pos, stat] = sum_r bc[r, off+pos] * smat[r, stat]
        ps_stats = psb_pool.tile([128, N_SCHUNK, 8], F32, tag='psb')
        for s in range(N_SCHUNK):
            off = S_CHUNK * s
            nc.tensor.matmul(
                ps_stats[:, s, :],
                bc[0:NR, off : off + 128],
                wp_sb[0:NR, NR : NR + 8],
                start=True, stop=True,
            )

        # epilogue: out = csum * exp(-0.5 * sqrt(ysum^2+xsum^2) / asum)
        # sqrt(s) = exp(0.5*ln(s)): Prelu/Square/Ln/Exp all live in the
        # natural_log_exp_and_others ACT table set -> no mid-kernel loads.
        rinv = tpool.tile([S_CHUNK, N_SCHUNK, NCLS], F32)
        sq = tpool.tile([S_CHUNK, N_SCHUNK, 4], F32)
        ssum = tpool.tile([S_CHUNK, N_SCHUNK, NCLS], F32)
        lns = tpool.tile([S_CHUNK, N_SCHUNK, NCLS], F32)
        srt = tpool.tile([S_CHUNK, N_SCHUNK, NCLS], F32)
        drift = tpool.tile([S_CHUNK, N_SCHUNK, NCLS], F32)
        expd = tpool.tile([S_CHUNK, N_SCHUNK, NCLS], F32)
        outv = tpool.tile([S_CHUNK, N_SCHUNK, NCLS], F32)
        nc.vector.reciprocal(rinv[:], ps_stats[0:S_CHUNK, :, 0:2])
        nc.scalar.activation(sq[:], ps_stats[0:S_CHUNK, :, 2:6], AF.Square)
        nc.vector.tensor_tensor(ssum[:], sq[:, :, 0:2], sq[:, :, 2:4], op=ALU.add)
        nc.scalar.activation(lns[:], ssum[:], AF.Ln)
        nc.scalar.activation(srt[:], lns[:], AF.Exp, scale=0.5)
        nc.vector.tensor_tensor(drift[:], srt[:], rinv[:], op=ALU.mult)
        nc.scalar.activation(expd[:], drift[:], AF.Exp, scale=-0.5)
        nc.vector.tensor_tensor(
            outv[:], ps_stats[0:S_CHUNK, :, 6:8], expd[:], op=ALU.mult
        )
        nc.sync.dma_start(out_d[:, 0 : NCLS * 14], outv[:, 0:14, :])
        nc.scalar.dma_start(out_d[:, NCLS * 14 :], outv[:, 14:, :])


def build_program():
    nc = bacc.Bacc("TRN2", target_bir_lowering=False, debug=False)
    x_d = nc.dram_tensor("x", [C, NPIX], BF16, kind="ExternalInput").ap()
    wp_d = nc.dram_tensor("wp", [C, NR + 8], BF16, kind="ExternalInput").ap()
    actp_d = nc.dram_tensor("actp", [NR, 2], F32, kind="ExternalInput").ap()
    out_d = nc.dram_tensor(
        "out", [S_CHUNK, N_SCHUNK * NCLS], F32, kind="ExternalOutput"
    ).ap()
    with tile.TileContext(nc) as tc:
        kernel_body(tc, x_d, wp_d, actp_d, out_d)
    nc.compile()
    return nc


def host_params(Wt, bias, Wlin, blin):
    """Fold conv weights + linear projection into device params."""
    Wt = np.asarray(Wt, np.float32)
    bias = np.asarray(bias, np.float32)
    Wlin = np.asarray(Wlin, np.float32)
    blin = np.asarray(blin, np.float32)
    O = Wt.shape[0]
    P25 = KS * KS
    Wp = Wt.reshape(O, C, P25)                        # (O, C, P)
    Weff = np.einsum("ko,ocp->kcp", Wlin, Wp)         # (2, C, P)
    beff = (Wlin @ bias + blin).astype(np.float32)    # (2,)
    offs = np.arange(-PAD, PAD + 1, dtype=np.float32)

    wp = np.zeros((C, NR + 8), np.float32)
    actp = np.zeros((NR, 2), np.float32)
    actp[0:AB, 1] = 1.0
    for p in range(P25):
        i, j = p // KS, p % KS
        for k in range(NCLS):
            rs = 32 * k + p            # sign row (u = k)
            rr = AB + 32 * k + p       # relu row (u = 2 + k)
            wp[:, rs] = Weff[k, :, p]
            wp[:, rr] = -Weff[k, :, p]     # negated: row holds -G'
            actp[rs, 0] = beff[k]
            actp[rr, 0] = -beff[k]
            # smat columns appended after the weight columns; smat row
            # index == ag row index. |v| = v + 2*relu(-v): sign rows carry
            # weight 1, relu rows weight 2, for the abs-based stats.
            wp[rs, NR + 6 + k] = 1.0           # csum (sign rows only)
            wp[rs, NR + 0 + k] = 1.0           # asum
            wp[rs, NR + 2 + k] = offs[i]       # ysum
            wp[rs, NR + 4 + k] = offs[j]       # xsum
            wp[rr, NR + 0 + k] = 2.0
            wp[rr, NR + 2 + k] = 2.0 * offs[i]
            wp[rr, NR + 4 + k] = 2.0 * offs[j]
    return wp.astype(ml_dtypes.bfloat16), actp


_nc_cache = None
last_results = None  # BassKernelResults of the most recent run (for profiling)


def kernel(x, Wt, bias, Wlin, blin):
    global _nc_cache, last_results
    x = np.asarray(x, np.float32)
    xpad = np.ascontiguousarray(
        np.pad(x, ((0, 0), (0, 0), (PAD, PAD), (PAD, PAD))).reshape(
            N_CORES, C, NPIX
        )
    ).astype(ml_dtypes.bfloat16)
    wp, actp = host_params(Wt, bias, Wlin, blin)
    if _nc_cache is None:
        _nc_cache = build_program()
    in_maps = [
        {"x": xpad[n], "wp": wp, "actp": actp} for n in range(N_CORES)
    ]
    res = run_bass_kernel_spmd(_nc_cache, in_maps, list(range(N_CORES)))
    last_results = res
    # device out: [120, 28, 2] = [(d, wp), h2, k]; pixel (h, w) lives at
    # partition 60*(h%2)+w, chunk h//2
    out = np.empty((N_CORES, H, W, NCLS), np.float32)
    hh = np.arange(H)
    for n in range(N_CORES):
        arr = res.results[n]["out"].reshape(S_CHUNK, N_SCHUNK, NCLS)
        # out[h, w, k] = arr[60*(h%2) + w, h//2, k]
        out[n] = arr[
            (60 * (hh % 2))[:, None] + np.arange(W)[None, :], (hh // 2)[:, None], :
        ]
    return out
